# revision 14
# baseline (speedup 1.0000x reference)
"""Trainium2 Bass kernel: BertCL mean-pool + NT-Xent contrastive loss.

Contract: kernel(last_hidden_states [256,512,768] f32, input_mask [256,512] f32)
-> scalar f32 loss, numerically matching the jax reference.

Strategy (8 NeuronCores, SPMD):
  Batch axis sharded STRIDED: core c owns logical batches {c, c+8, c+16, ...}
  (local j <-> logical c + 8j), so an all-gather of locals [j0,j1) delivers
  the contiguous block of logical batches [8*j0, 8*j1).

  stage 1 (memory-bound): per local batch, stream [512,768] through SBUF as a
    [128, 4*768] float32r tile and reduce the sequence axis with ones-vector
    fp32r matmuls (1 PE cycle/row at >=256-wide output vs 4 for fp32)
    accumulating in PSUM -> [1,768] sums staged into one SBUF row, then
    DMA'd per batch into cc_in on the ACT HWDGE queue (so the final
    collective's input never waits behind the big SP-queue input stream).
  Three asymmetric AllGathers of the raw sums (the reference's division by
    the mask row-sum cancels exactly in the L2 normalization, so it is
    skipped): locals [0,16) at b=15 and [16,24) at b=23 are fully hidden
    under the remaining input streaming; only the small final gather of
    locals [24,32) (64 logical rows) is exposed. After each gather the core
    L2-normalizes the block (1/tau folded into the norm), transposes it via
    PE into zT, and accumulates the logits block S[0:64, block].
  Finish, split around the final gather: the masked exp+accum over columns
    [0,192) and the strict-upper-triangle pair reduction run inside the
    final collective's latency window; after the last logits block only
    exp+accum over [64,64] straight from PSUM (no diagonal there), the add,
    ln, and a single fused dot  [ld; rs] . [cnt; -1]  remain, then scale
    and the output DMA. exp without max-subtraction is safe: logits are
    cosines/tau in [-2,2].

  Measured (paired K-differential, see perf_lab.py): baseline fp32 was
  ~230us; fp32r pooling cut it to ~154us; the asymmetric-gather tail
  restructure + split finish to ~132-138us vs the ~116us measured
  stage-1 HBM floor (~434 GB/s effective per core; the remaining ~16-22us
  is the final collective's constant latency plus a ~6us finish chain).
  Rejected by measurement: 12KB DMA descriptors (128.3us s1 vs 116.3us
  with 4x3KB strided - small interleaved descriptors spread better across
  HBM), striping the input stream across both HWDGE queues (119.9us s1 -
  HBM-limited, not queue-limited), replacing AllGathers with local-copy
  fan-out (slower + noisy). Relative error vs the fp32 jax reference:
  4.4e-7 on hardware.

  NOTE: fused DVE ops (tensor_tensor_reduce, scalar_tensor_tensor) pass
  CoreSim but hang/crash this hardware - only plain DVE ops are used.
"""

import sys
from contextlib import ExitStack

import numpy as np

_REPO = "/opt/trn_rl_repo"
if _REPO not in sys.path:
    sys.path.insert(0, _REPO)

import concourse.bass as bass  # noqa: E402  (kept for callers/debugging)
import concourse.tile as tile  # noqa: E402
from concourse import bacc, bass_utils, mybir  # noqa: E402

N_CORES = 8
B, S, H = 256, 512, 768
B_SH = B // N_CORES  # 32 local batches per core
N_PAIR = B // 4  # 64
TAU = 0.5
F32 = mybir.dt.float32
F32R = mybir.dt.float32r  # PE fast-fp32 mode: 1 cycle/row at >=256-wide out
X_DT = F32R  # dtype of the streamed input (np binding is float32 either way)
AX = mybir.AxisListType
AF = mybir.ActivationFunctionType
NEG = -30000.0  # diagonal mask value; exp(NEG + logit) == 0 exactly in fp32

# gather segments over local batch indices; the last one is small so the
# only exposed collective carries just 64 logical rows
SEG = [(0, 16), (16, 24), (24, 32)]


def _body(
    tc,
    x,
    ident,
    dmask,
    triu,
    cw,
    out,
    use_collective=True,
    stages=("s1", "cc", "s2"),
):
    nc = tc.nc

    with ExitStack() as ctx:
        const = ctx.enter_context(tc.tile_pool(name="const", bufs=1))
        ones_col = const.tile([128, 1], F32)
        nc.vector.memset(ones_col[:], 1.0)
        idt = const.tile([128, 128], F32)
        nc.scalar.dma_start(idt[:], ident[:])

        dram = ctx.enter_context(tc.tile_pool(name="dram", bufs=1, space="DRAM"))
        cc_in = dram.tile([B_SH, H], F32)
        shared = "Shared" if use_collective else "Local"
        cc_o = [
            dram.tile([8 * (j1 - j0), H], F32, addr_space=shared, name=f"cc_o{h}")
            for h, (j0, j1) in enumerate(SEG)
        ]

        # staging row for pooled sums: [1, 32*768] on partition 0
        pooled_sb = const.tile([1, B_SH * H], F32)

        xin = ctx.enter_context(tc.tile_pool(name="xin", bufs=6))
        ps1 = ctx.enter_context(tc.tile_pool(name="ps1", bufs=2, space="PSUM"))
        s2 = ctx.enter_context(tc.tile_pool(name="s2", bufs=1))
        s2t = ctx.enter_context(tc.tile_pool(name="s2t", bufs=2))
        psT = ctx.enter_context(tc.tile_pool(name="psT", bufs=2, space="PSUM"))
        psS = ctx.enter_context(tc.tile_pool(name="psS", bufs=1, space="PSUM"))

        # zT[:, k*256 + p] = z[p, k*128 + q] for partition q (h on partitions)
        zT = s2.tile([128, 6 * B], F32)
        pS = psS.tile([N_PAIR, B], F32)

        def send_seg(h):
            """AllGather raw pooled sums for local rows [SEG[h]) (staged
            per-batch into cc_in by the loop below)."""
            j0, j1 = SEG[h]
            if use_collective:
                nc.gpsimd.collective_compute(
                    "AllGather",
                    mybir.AluOpType.bypass,
                    replica_groups=[list(range(N_CORES))],
                    ins=[cc_in[j0:j1, :].opt()],
                    outs=[cc_o[h].opt()],
                )
            else:
                n = j1 - j0
                for c in range(N_CORES):
                    nc.sync.dma_start(
                        cc_o[h][c * n : (c + 1) * n, :], cc_in[j0:j1, :]
                    )

        def consume_block(h, ja, jb, name):
            """Normalize logical rows [8*ja, 8*jb) from gather h; fill zT cols.

            Gathered row (c, j - SEG[h][0]) holds logical batch c + 8j; the
            permuted 3-D AP (j, c, e) lands partitions in logical order."""
            P = 8 * (jb - ja)  # rows in this block
            col = 8 * ja  # zT column base = first logical row
            zh = s2.tile([P, H], F32, tag=name, name=name)
            src = cc_o[h].rearrange("(c j) e -> j c e", c=N_CORES)
            nc.scalar.dma_start(zh[:], src[ja - SEG[h][0] : jb - SEG[h][0]])
            sqs = s2t.tile([P, H], F32, tag=f"sqs{name}", name=f"sqs{name}")
            ssn = s2t.tile([P, 1], F32, tag=f"ssn{name}", name=f"ssn{name}")
            nc.vector.tensor_mul(sqs[:], zh[:], zh[:])
            nc.vector.reduce_sum(out=ssn[:], in_=sqs[:], axis=AX.X)
            # sqrt(TAU * ss): scales z by 1/sqrt(tau) so S = z'z'^T = logits
            nrm = s2t.tile([P, 1], F32, tag=f"nrm{name}", name=f"nrm{name}")
            nc.scalar.activation(nrm[:], ssn[:], AF.Sqrt, scale=TAU)
            rn = s2t.tile([P, 1], F32, tag=f"rn{name}", name=f"rn{name}")
            nc.vector.reciprocal(rn[:], nrm[:])
            nc.vector.tensor_scalar_mul(zh[:], zh[:], rn[:, 0:1])
            for k in range(6):
                pt = psT.tile([128, 128], F32, tag="pt")
                nc.tensor.transpose(
                    pt[:, 0:P], zh[:, k * 128 : (k + 1) * 128], idt[0:P, 0:P]
                )
                nc.vector.tensor_copy(
                    zT[:, k * B + col : k * B + col + P], pt[:, 0:P]
                )

        def logits_block(col, n):
            """S[0:64, col:col+n] += sum_k zT_k[:, 0:64].T @ zT_k[:, col:col+n]"""
            for k in range(6):
                nc.tensor.matmul(
                    pS[:, col : col + n],
                    lhsT=zT[:, k * B : k * B + N_PAIR],
                    rhs=zT[:, k * B + col : k * B + col + n],
                    start=(k == 0),
                    stop=(k == 5),
                )

        # ---- stage 1: per-batch sum over the sequence axis -------------------
        # partition p holds seq rows {c*128+p}: 4x 3KB DMA descriptors per
        # partition. (Measured FASTER than one 12KB descriptor per partition
        # - the smaller interleaved pattern spreads better across HBM.)
        x4 = x.rearrange("b (c p) e -> b p c e", p=128)  # [32, 128, 4, 768]
        for b in range(B_SH):
            if "s1" in stages:
                xt = xin.tile([128, 4 * H], F32R)
                nc.sync.dma_start(xt[:], x4[b])
                ps = ps1.tile([1, H], F32)
                for c in range(4):
                    nc.tensor.matmul(
                        ps[:, 0:512],
                        lhsT=ones_col[:, 0:1].bitcast(F32R),
                        rhs=xt[:, c * H : c * H + 512],
                        start=(c == 0),
                        stop=(c == 3),
                    )
                for c in range(4):
                    nc.tensor.matmul(
                        ps[:, 512:H],
                        lhsT=ones_col[:, 0:1].bitcast(F32R),
                        rhs=xt[:, c * H + 512 : (c + 1) * H],
                        start=(c == 0),
                        stop=(c == 3),
                    )
                nc.scalar.copy(pooled_sb[0:1, b * H : (b + 1) * H], ps[:])
            if "cc" in stages:
                # per-batch staging on the ACT HWDGE queue: never queued
                # behind the big SP-queue x stream
                nc.scalar.dma_start(
                    cc_in[b : b + 1, :], pooled_sb[0:1, b * H : (b + 1) * H]
                )
                for h, (j0, j1) in enumerate(SEG):
                    if b == j1 - 1:
                        send_seg(h)

        if "cc" not in stages or "s2" not in stages:
            return

        # ---- consume gathers 0,1 (hidden in the final gather's window) ------
        consume_block(0, 0, 16, "zb0")
        logits_block(0, 128)
        consume_block(1, 16, 24, "zb1")
        logits_block(128, 64)

        # ---- early finish: everything not needing columns [192,256) ---------
        # uv stacks [ld; rs] so one dot against cw = [cnt; -1] finishes it
        uv = s2.tile([128, 1], F32)
        dm = s2.tile([N_PAIR, 192], F32)
        nc.scalar.dma_start(dm[:], dmask[:, 0:192])
        sd0 = s2.tile([N_PAIR, 192], F32)
        nc.vector.tensor_add(sd0[:], pS[:, 0:192], dm[:])
        et0 = s2.tile([N_PAIR, 192], F32)
        se0 = s2.tile([N_PAIR, 1], F32)
        nc.scalar.activation(et0[:], sd0[:], AF.Exp, scale=1.0, accum_out=se0[:])
        tri_t = s2.tile([N_PAIR, N_PAIR], F32)
        nc.scalar.dma_start(tri_t[:], triu[:])
        mt2 = s2.tile([N_PAIR, N_PAIR], F32)
        nc.vector.tensor_mul(mt2[:], sd0[0:N_PAIR, 0:N_PAIR], tri_t[:])
        rs = s2.tile([N_PAIR, 1], F32)
        nc.vector.reduce_sum(out=rs[:], in_=mt2[:], axis=AX.X)
        # partition-shift rs into the bottom half of uv (SBUF->SBUF DMA)
        nc.scalar.dma_start(uv[N_PAIR : 2 * N_PAIR, 0:1], rs[:])
        cw_t = s2.tile([128, 1], F32)
        nc.scalar.dma_start(cw_t[:], cw[:])

        # ---- exposed tail: final gather block + short chain -----------------
        consume_block(2, 24, 32, "zb2")
        logits_block(192, 64)
        # no diagonal in columns [192,256): exp straight from PSUM
        et1 = s2.tile([N_PAIR, 64], F32)
        se1 = s2.tile([N_PAIR, 1], F32)
        nc.scalar.activation(
            et1[:], pS[:, 192:256], AF.Exp, scale=1.0, accum_out=se1[:]
        )
        # logden = ln(se1 + se0): bias-AP fusion keeps the add off the tail
        nc.scalar.activation(uv[0:N_PAIR, :], se1[:], AF.Ln, bias=se0[:, 0:1])
        ptot = psS.tile([1, 1], F32, tag="ptot")
        nc.tensor.matmul(ptot[:], lhsT=uv[:], rhs=cw_t[:], start=True, stop=True)
        res = s2.tile([1, 1], F32)
        nc.vector.tensor_scalar_mul(res[:], ptot[:], -2.0 / N_PAIR * (N_PAIR - 1))
        nc.scalar.dma_start(out[0:1, 0:1], res[:])


def build_nc():
    nc = bacc.Bacc("TRN2", target_bir_lowering=False, debug=False, num_devices=N_CORES)
    x = nc.dram_tensor("x", [B_SH, S, H], X_DT, kind="ExternalInput")
    ident = nc.dram_tensor("ident", [128, 128], F32, kind="ExternalInput")
    dmask = nc.dram_tensor("dmask", [N_PAIR, B], F32, kind="ExternalInput")
    triu = nc.dram_tensor("triu", [N_PAIR, N_PAIR], F32, kind="ExternalInput")
    cw = nc.dram_tensor("cw", [128, 1], F32, kind="ExternalInput")
    out = nc.dram_tensor("loss", [1, 1], F32, kind="ExternalOutput")
    with tile.TileContext(nc) as tc:
        _body(
            tc,
            x.ap(),
            ident.ap(),
            dmask.ap(),
            triu.ap(),
            cw.ap(),
            out.ap(),
        )
    nc.compile()
    return nc


def const_inputs():
    ident = np.eye(128, dtype=np.float32)
    dmask = np.zeros((N_PAIR, B), dtype=np.float32)
    dmask[np.arange(N_PAIR), np.arange(N_PAIR)] = NEG
    triu = np.triu(np.ones((N_PAIR, N_PAIR), dtype=np.float32), k=1)
    cw = np.concatenate(
        [
            (N_PAIR - 1 - np.arange(N_PAIR, dtype=np.float32)),  # cnt_i
            -np.ones(N_PAIR, dtype=np.float32),
        ]
    ).reshape(128, 1)
    return {"ident": ident, "dmask": dmask, "triu": triu, "cw": cw}


def make_in_maps(last_hidden_states, input_mask):
    del input_mask  # cancels exactly in the L2 normalization (see send_seg)
    x = np.asarray(last_hidden_states, dtype=np.float32)
    consts = const_inputs()
    return [
        {"x": np.ascontiguousarray(x[c::N_CORES]), **consts}  # logical c+8j
        for c in range(N_CORES)
    ]


_CACHE = {}


def get_nc():
    if "nc" not in _CACHE:
        _CACHE["nc"] = build_nc()
    return _CACHE["nc"]


def kernel(last_hidden_states, input_mask):
    nc = get_nc()
    in_maps = make_in_maps(last_hidden_states, input_mask)
    res = bass_utils.run_bass_kernel_spmd(nc, in_maps, core_ids=list(range(N_CORES)))
    return np.asarray(res.results[0]["loss"], dtype=np.float32).reshape(())


# revision 15
# speedup vs baseline: 1.0903x; 1.0903x over previous
"""Trainium2 Bass kernel: BertCL mean-pool + NT-Xent contrastive loss.

Contract: kernel(last_hidden_states [256,512,768] f32, input_mask [256,512] f32)
-> scalar f32 loss, numerically matching the jax reference.

Strategy (8 NeuronCores, SPMD):
  Batch axis sharded STRIDED: core c owns logical batches {c, c+8, c+16, ...}
  (local j <-> logical c + 8j), so an all-gather of locals [j0,j1) delivers
  the contiguous block of logical batches [8*j0, 8*j1).

  stage 1 (memory-bound): per local batch, stream [512,768] through SBUF as a
    [128, 4*768] float32r tile and reduce the sequence axis with ones-vector
    fp32r matmuls (1 PE cycle/row at >=256-wide output vs 4 for fp32)
    accumulating in PSUM -> [1,768] sums staged into one SBUF row, then
    DMA'd per batch into cc_in on the ACT HWDGE queue (so the final
    collective's input never waits behind the big SP-queue input stream).
  Three asymmetric AllGathers of the raw sums (the reference's division by
    the mask row-sum cancels exactly in the L2 normalization, so it is
    skipped): locals [0,16) at b=15 and [16,24) at b=23 are fully hidden
    under the remaining input streaming; only the small final gather of
    locals [24,32) (64 logical rows) is exposed. After each gather the core
    L2-normalizes the block (1/tau folded into the norm), transposes it via
    PE into zT, and accumulates the logits block S[0:64, block].
  Finish, split around the final gather: the masked exp+accum over columns
    [0,192) and the strict-upper-triangle pair reduction run inside the
    final collective's latency window; after the last logits block only
    exp+accum over [64,64] straight from PSUM (no diagonal there), the add,
    ln, and a single fused dot  [ld; rs] . [cnt; -1]  remain, then scale
    and the output DMA. exp without max-subtraction is safe: logits are
    cosines/tau in [-2,2].

  Measured (paired K-differential, see perf_lab.py): baseline fp32 was
  ~230us; fp32r pooling cut it to ~154us; the asymmetric-gather tail
  restructure + split finish to ~132-138us vs the ~116us measured
  stage-1 HBM floor (~434 GB/s effective per core; the remaining ~16-22us
  is the final collective's constant latency plus a ~6us finish chain).
  Rejected by measurement: 12KB DMA descriptors (128.3us s1 vs 116.3us
  with 4x3KB strided - small interleaved descriptors spread better across
  HBM), striping the input stream across both HWDGE queues (119.9us s1 -
  HBM-limited, not queue-limited), replacing AllGathers with local-copy
  fan-out (slower + noisy). Relative error vs the fp32 jax reference:
  4.4e-7 on hardware.

  NOTE: fused DVE ops (tensor_tensor_reduce, scalar_tensor_tensor) pass
  CoreSim but hang/crash this hardware - only plain DVE ops are used.
"""

import sys
from contextlib import ExitStack

import numpy as np

_REPO = "/opt/trn_rl_repo"
if _REPO not in sys.path:
    sys.path.insert(0, _REPO)

import concourse.bass as bass  # noqa: E402  (kept for callers/debugging)
import concourse.tile as tile  # noqa: E402
from concourse import bacc, bass_utils, mybir  # noqa: E402

N_CORES = 8
B, S, H = 256, 512, 768
B_SH = B // N_CORES  # 32 local batches per core
N_PAIR = B // 4  # 64
TAU = 0.5
F32 = mybir.dt.float32
F32R = mybir.dt.float32r  # PE fast-fp32 mode: 1 cycle/row at >=256-wide out
X_DT = F32R  # dtype of the streamed input (np binding is float32 either way)
AX = mybir.AxisListType
AF = mybir.ActivationFunctionType
NEG = -30000.0  # diagonal mask value; exp(NEG + logit) == 0 exactly in fp32

# gather segments over local batch indices; the last one is small so the
# only exposed collective carries just 64 logical rows
SEG = [(0, 16), (16, 24), (24, 32)]


def _body(
    tc,
    x,
    ident,
    dmask,
    triu,
    cw,
    out,
    use_collective=True,
    stages=("s1", "cc", "s2"),
):
    nc = tc.nc

    with ExitStack() as ctx:
        const = ctx.enter_context(tc.tile_pool(name="const", bufs=1))
        ones_col = const.tile([128, 1], F32)
        nc.vector.memset(ones_col[:], 1.0)
        idt = const.tile([128, 128], F32)
        nc.scalar.dma_start(idt[:], ident[:])

        dram = ctx.enter_context(tc.tile_pool(name="dram", bufs=1, space="DRAM"))
        cc_in = dram.tile([B_SH, H], F32)
        shared = "Shared" if use_collective else "Local"
        cc_o = [
            dram.tile([8 * (j1 - j0), H], F32, addr_space=shared, name=f"cc_o{h}")
            for h, (j0, j1) in enumerate(SEG)
        ]

        # staging row for pooled sums: [1, 32*768] on partition 0
        pooled_sb = const.tile([1, B_SH * H], F32)

        xin = ctx.enter_context(tc.tile_pool(name="xin", bufs=6))
        ps1 = ctx.enter_context(tc.tile_pool(name="ps1", bufs=2, space="PSUM"))
        s2 = ctx.enter_context(tc.tile_pool(name="s2", bufs=1))
        s2t = ctx.enter_context(tc.tile_pool(name="s2t", bufs=2))
        psT = ctx.enter_context(tc.tile_pool(name="psT", bufs=2, space="PSUM"))
        psS = ctx.enter_context(tc.tile_pool(name="psS", bufs=1, space="PSUM"))

        # zT[:, k*256 + p] = z[p, k*128 + q] for partition q (h on partitions)
        zT = s2.tile([128, 6 * B], F32)
        pS = psS.tile([N_PAIR, B], F32)

        def send_seg(h):
            """AllGather raw pooled sums for local rows [SEG[h]) (staged
            per-batch into cc_in by the loop below)."""
            j0, j1 = SEG[h]
            if use_collective:
                nc.gpsimd.collective_compute(
                    "AllGather",
                    mybir.AluOpType.bypass,
                    replica_groups=[list(range(N_CORES))],
                    ins=[cc_in[j0:j1, :].opt()],
                    outs=[cc_o[h].opt()],
                )
            else:
                n = j1 - j0
                for c in range(N_CORES):
                    nc.sync.dma_start(
                        cc_o[h][c * n : (c + 1) * n, :], cc_in[j0:j1, :]
                    )

        def consume_block(h, ja, jb, name):
            """Normalize logical rows [8*ja, 8*jb) from gather h; fill zT cols.

            Gathered row (c, j - SEG[h][0]) holds logical batch c + 8j; the
            permuted 3-D AP (j, c, e) lands partitions in logical order."""
            P = 8 * (jb - ja)  # rows in this block
            col = 8 * ja  # zT column base = first logical row
            zh = s2.tile([P, H], F32, tag=name, name=name)
            src = cc_o[h].rearrange("(c j) e -> j c e", c=N_CORES)
            nc.gpsimd.dma_start(zh[:], src[ja - SEG[h][0] : jb - SEG[h][0]])
            sqs = s2t.tile([P, H], F32, tag=f"sqs{name}", name=f"sqs{name}")
            ssn = s2t.tile([P, 1], F32, tag=f"ssn{name}", name=f"ssn{name}")
            nc.vector.tensor_mul(sqs[:], zh[:], zh[:])
            nc.vector.reduce_sum(out=ssn[:], in_=sqs[:], axis=AX.X)
            # sqrt(TAU * ss): scales z by 1/sqrt(tau) so S = z'z'^T = logits
            nrm = s2t.tile([P, 1], F32, tag=f"nrm{name}", name=f"nrm{name}")
            nc.scalar.activation(nrm[:], ssn[:], AF.Sqrt, scale=TAU)
            rn = s2t.tile([P, 1], F32, tag=f"rn{name}", name=f"rn{name}")
            nc.vector.reciprocal(rn[:], nrm[:])
            nc.vector.tensor_scalar_mul(zh[:], zh[:], rn[:, 0:1])
            for k in range(6):
                pt = psT.tile([128, 128], F32, tag="pt")
                nc.tensor.transpose(
                    pt[:, 0:P], zh[:, k * 128 : (k + 1) * 128], idt[0:P, 0:P]
                )
                nc.vector.tensor_copy(
                    zT[:, k * B + col : k * B + col + P], pt[:, 0:P]
                )

        def logits_block(col, n):
            """S[0:64, col:col+n] += sum_k zT_k[:, 0:64].T @ zT_k[:, col:col+n]"""
            for k in range(6):
                nc.tensor.matmul(
                    pS[:, col : col + n],
                    lhsT=zT[:, k * B : k * B + N_PAIR],
                    rhs=zT[:, k * B + col : k * B + col + n],
                    start=(k == 0),
                    stop=(k == 5),
                )

        # ---- stage 1: per-batch sum over the sequence axis -------------------
        # partition p holds seq rows {c*128+p}: 4x 3KB DMA descriptors per
        # partition. (Measured FASTER than one 12KB descriptor per partition
        # - the smaller interleaved pattern spreads better across HBM.)
        x4 = x.rearrange("b (c p) e -> b p c e", p=128)  # [32, 128, 4, 768]
        for b in range(B_SH):
            if "s1" in stages:
                xt = xin.tile([128, 4 * H], F32R)
                nc.sync.dma_start(xt[:], x4[b])
                ps = ps1.tile([1, H], F32)
                for c in range(4):
                    nc.tensor.matmul(
                        ps[:, 0:512],
                        lhsT=ones_col[:, 0:1].bitcast(F32R),
                        rhs=xt[:, c * H : c * H + 512],
                        start=(c == 0),
                        stop=(c == 3),
                    )
                for c in range(4):
                    nc.tensor.matmul(
                        ps[:, 512:H],
                        lhsT=ones_col[:, 0:1].bitcast(F32R),
                        rhs=xt[:, c * H + 512 : (c + 1) * H],
                        start=(c == 0),
                        stop=(c == 3),
                    )
                nc.scalar.copy(pooled_sb[0:1, b * H : (b + 1) * H], ps[:])
            if "cc" in stages:
                # per-batch staging on the ACT HWDGE queue: never queued
                # behind the big SP-queue x stream
                nc.scalar.dma_start(
                    cc_in[b : b + 1, :], pooled_sb[0:1, b * H : (b + 1) * H]
                )
                for h, (j0, j1) in enumerate(SEG):
                    if b == j1 - 1:
                        send_seg(h)

        if "cc" not in stages or "s2" not in stages:
            return

        # ---- consume gathers 0,1 (hidden in the final gather's window) ------
        consume_block(0, 0, 16, "zb0")
        logits_block(0, 128)
        consume_block(1, 16, 24, "zb1")
        logits_block(128, 64)

        # ---- early finish: everything not needing columns [192,256) ---------
        # uv stacks [ld; rs] so one dot against cw = [cnt; -1] finishes it
        uv = s2.tile([128, 1], F32)
        dm = s2.tile([N_PAIR, 192], F32)
        nc.scalar.dma_start(dm[:], dmask[:, 0:192])
        sd0 = s2.tile([N_PAIR, 192], F32)
        nc.vector.tensor_add(sd0[:], pS[:, 0:192], dm[:])
        et0 = s2.tile([N_PAIR, 192], F32)
        se0 = s2.tile([N_PAIR, 1], F32)
        nc.scalar.activation(et0[:], sd0[:], AF.Exp, scale=1.0, accum_out=se0[:])
        tri_t = s2.tile([N_PAIR, N_PAIR], F32)
        nc.scalar.dma_start(tri_t[:], triu[:])
        mt2 = s2.tile([N_PAIR, N_PAIR], F32)
        nc.vector.tensor_mul(mt2[:], sd0[0:N_PAIR, 0:N_PAIR], tri_t[:])
        rs = s2.tile([N_PAIR, 1], F32)
        nc.vector.reduce_sum(out=rs[:], in_=mt2[:], axis=AX.X)
        # partition-shift rs into the bottom half of uv (SBUF->SBUF DMA)
        nc.gpsimd.dma_start(uv[N_PAIR : 2 * N_PAIR, 0:1], rs[:])
        cw_t = s2.tile([128, 1], F32)
        nc.scalar.dma_start(cw_t[:], cw[:])

        # ---- exposed tail: final gather block + short chain -----------------
        consume_block(2, 24, 32, "zb2")
        logits_block(192, 64)
        # no diagonal in columns [192,256): exp straight from PSUM
        et1 = s2.tile([N_PAIR, 64], F32)
        se1 = s2.tile([N_PAIR, 1], F32)
        nc.scalar.activation(
            et1[:], pS[:, 192:256], AF.Exp, scale=1.0, accum_out=se1[:]
        )
        # logden = ln(se1 + se0): bias-AP fusion keeps the add off the tail
        nc.scalar.activation(uv[0:N_PAIR, :], se1[:], AF.Ln, bias=se0[:, 0:1])
        ptot = psS.tile([1, 1], F32, tag="ptot")
        nc.tensor.matmul(ptot[:], lhsT=uv[:], rhs=cw_t[:], start=True, stop=True)
        res = s2.tile([1, 1], F32)
        nc.vector.tensor_scalar_mul(res[:], ptot[:], -2.0 / N_PAIR * (N_PAIR - 1))
        nc.gpsimd.dma_start(out[0:1, 0:1], res[:])


def build_nc():
    nc = bacc.Bacc("TRN2", target_bir_lowering=False, debug=False, num_devices=N_CORES)
    x = nc.dram_tensor("x", [B_SH, S, H], X_DT, kind="ExternalInput")
    ident = nc.dram_tensor("ident", [128, 128], F32, kind="ExternalInput")
    dmask = nc.dram_tensor("dmask", [N_PAIR, B], F32, kind="ExternalInput")
    triu = nc.dram_tensor("triu", [N_PAIR, N_PAIR], F32, kind="ExternalInput")
    cw = nc.dram_tensor("cw", [128, 1], F32, kind="ExternalInput")
    out = nc.dram_tensor("loss", [1, 1], F32, kind="ExternalOutput")
    with tile.TileContext(nc) as tc:
        _body(
            tc,
            x.ap(),
            ident.ap(),
            dmask.ap(),
            triu.ap(),
            cw.ap(),
            out.ap(),
        )
    nc.compile()
    return nc


def const_inputs():
    ident = np.eye(128, dtype=np.float32)
    dmask = np.zeros((N_PAIR, B), dtype=np.float32)
    dmask[np.arange(N_PAIR), np.arange(N_PAIR)] = NEG
    triu = np.triu(np.ones((N_PAIR, N_PAIR), dtype=np.float32), k=1)
    cw = np.concatenate(
        [
            (N_PAIR - 1 - np.arange(N_PAIR, dtype=np.float32)),  # cnt_i
            -np.ones(N_PAIR, dtype=np.float32),
        ]
    ).reshape(128, 1)
    return {"ident": ident, "dmask": dmask, "triu": triu, "cw": cw}


def make_in_maps(last_hidden_states, input_mask):
    del input_mask  # cancels exactly in the L2 normalization (see send_seg)
    x = np.asarray(last_hidden_states, dtype=np.float32)
    consts = const_inputs()
    return [
        {"x": np.ascontiguousarray(x[c::N_CORES]), **consts}  # logical c+8j
        for c in range(N_CORES)
    ]


_CACHE = {}


def get_nc():
    if "nc" not in _CACHE:
        _CACHE["nc"] = build_nc()
    return _CACHE["nc"]


def kernel(last_hidden_states, input_mask):
    nc = get_nc()
    in_maps = make_in_maps(last_hidden_states, input_mask)
    res = bass_utils.run_bass_kernel_spmd(nc, in_maps, core_ids=list(range(N_CORES)))
    return np.asarray(res.results[0]["loss"], dtype=np.float32).reshape(())


# revision 16
# speedup vs baseline: 1.1359x; 1.0418x over previous
"""Trainium2 Bass kernel: BertCL mean-pool + NT-Xent contrastive loss.

Contract: kernel(last_hidden_states [256,512,768] f32, input_mask [256,512] f32)
-> scalar f32 loss, numerically matching the jax reference.

Strategy (8 NeuronCores, SPMD):
  Batch axis sharded STRIDED: core c owns logical batches {c, c+8, c+16, ...}
  (local j <-> logical c + 8j), so an all-gather of locals [j0,j1) delivers
  the contiguous block of logical batches [8*j0, 8*j1).

  stage 1 (memory-bound): per local batch, stream [512,768] through SBUF as a
    [128, 4*768] float32r tile and reduce the sequence axis with ones-vector
    fp32r matmuls (1 PE cycle/row at >=256-wide output vs 4 for fp32)
    accumulating in PSUM -> [1,768] sums staged into one SBUF row, then
    DMA'd per batch into cc_in on the ACT HWDGE queue (so the final
    collective's input never waits behind the big SP-queue input stream).
  Three asymmetric AllGathers of the raw sums (the reference's division by
    the mask row-sum cancels exactly in the L2 normalization, so it is
    skipped): locals [0,16) at b=15 and [16,24) at b=23 are fully hidden
    under the remaining input streaming; only the small final gather of
    locals [24,32) (64 logical rows) is exposed. After each gather the core
    L2-normalizes the block (1/tau folded into the norm), transposes it via
    PE into zT, and accumulates the logits block S[0:64, block].
  Finish, split around the final gather: the masked exp+accum over columns
    [0,192) and the strict-upper-triangle pair reduction run inside the
    final collective's latency window; after the last logits block only
    exp+accum over [64,64] straight from PSUM (no diagonal there), the add,
    ln, and a single fused dot  [ld; rs] . [cnt; -1]  remain, then scale
    and the output DMA. exp without max-subtraction is safe: logits are
    cosines/tau in [-2,2].

  Measured (paired K-differential, see perf_lab.py): baseline fp32 was
  ~230us; fp32r pooling cut it to ~154us; the asymmetric-gather tail
  restructure + split finish to ~132-138us vs the ~116us measured
  stage-1 HBM floor (~434 GB/s effective per core; the remaining ~16-22us
  is the final collective's constant latency plus a ~6us finish chain).
  Rejected by measurement: 12KB DMA descriptors (128.3us s1 vs 116.3us
  with 4x3KB strided - small interleaved descriptors spread better across
  HBM), striping the input stream across both HWDGE queues (119.9us s1 -
  HBM-limited, not queue-limited), replacing AllGathers with local-copy
  fan-out (slower + noisy). Relative error vs the fp32 jax reference:
  4.4e-7 on hardware.

  NOTE: fused DVE ops (tensor_tensor_reduce, scalar_tensor_tensor) pass
  CoreSim but hang/crash this hardware - only plain DVE ops are used.
"""

import sys
from contextlib import ExitStack

import numpy as np

_REPO = "/opt/trn_rl_repo"
if _REPO not in sys.path:
    sys.path.insert(0, _REPO)

import concourse.bass as bass  # noqa: E402  (kept for callers/debugging)
import concourse.tile as tile  # noqa: E402
from concourse import bacc, bass_utils, mybir  # noqa: E402

N_CORES = 8
B, S, H = 256, 512, 768
B_SH = B // N_CORES  # 32 local batches per core
N_PAIR = B // 4  # 64
TAU = 0.5
F32 = mybir.dt.float32
F32R = mybir.dt.float32r  # PE fast-fp32 mode: 1 cycle/row at >=256-wide out
X_DT = F32R  # dtype of the streamed input (np binding is float32 either way)
AX = mybir.AxisListType
AF = mybir.ActivationFunctionType
NEG = -30000.0  # diagonal mask value; exp(NEG + logit) == 0 exactly in fp32

# gather segments over local batch indices; the last one is small so the
# only exposed collective carries just 64 logical rows
SEG = [(0, 16), (16, 24), (24, 32)]


def _body(
    tc,
    x,
    ident,
    dmask,
    triu,
    cw,
    out,
    use_collective=True,
    stages=("s1", "cc", "s2"),
):
    nc = tc.nc

    with ExitStack() as ctx:
        const = ctx.enter_context(tc.tile_pool(name="const", bufs=1))
        ones_col = const.tile([128, 1], F32)
        nc.vector.memset(ones_col[:], 1.0)
        idt = const.tile([128, 128], F32)
        nc.scalar.dma_start(idt[:], ident[:])

        dram = ctx.enter_context(tc.tile_pool(name="dram", bufs=1, space="DRAM"))
        cc_in = dram.tile([B_SH, H], F32)
        shared = "Shared" if use_collective else "Local"
        cc_o = [
            dram.tile([8 * (j1 - j0), H], F32, addr_space=shared, name=f"cc_o{h}")
            for h, (j0, j1) in enumerate(SEG)
        ]

        # staging row for pooled sums: [1, 32*768] on partition 0
        pooled_sb = const.tile([1, B_SH * H], F32)

        xin = ctx.enter_context(tc.tile_pool(name="xin", bufs=6))
        ps1 = ctx.enter_context(tc.tile_pool(name="ps1", bufs=2, space="PSUM"))
        s2 = ctx.enter_context(tc.tile_pool(name="s2", bufs=1))
        s2t = ctx.enter_context(tc.tile_pool(name="s2t", bufs=2))
        psT = ctx.enter_context(tc.tile_pool(name="psT", bufs=2, space="PSUM"))
        psS = ctx.enter_context(tc.tile_pool(name="psS", bufs=1, space="PSUM"))

        # zT[:, k*256 + p] = z[p, k*128 + q] for partition q (h on partitions)
        zT = s2.tile([128, 6 * B], F32)
        pS = psS.tile([N_PAIR, B], F32)

        def send_seg(h):
            """AllGather raw pooled sums for local rows [SEG[h]) (staged
            per-batch into cc_in by the loop below)."""
            j0, j1 = SEG[h]
            if use_collective:
                nc.gpsimd.collective_compute(
                    "AllGather",
                    mybir.AluOpType.bypass,
                    replica_groups=[list(range(N_CORES))],
                    ins=[cc_in[j0:j1, :].opt()],
                    outs=[cc_o[h].opt()],
                )
            else:
                n = j1 - j0
                for c in range(N_CORES):
                    nc.sync.dma_start(
                        cc_o[h][c * n : (c + 1) * n, :], cc_in[j0:j1, :]
                    )

        def consume_block(h, ja, jb, name, dma_eng=None):
            """Normalize logical rows [8*ja, 8*jb) from gather h; fill zT cols.

            Gathered row (c, j - SEG[h][0]) holds logical batch c + 8j; the
            permuted 3-D AP (j, c, e) lands partitions in logical order."""
            P = 8 * (jb - ja)  # rows in this block
            col = 8 * ja  # zT column base = first logical row
            zh = s2.tile([P, H], F32, tag=name, name=name)
            src = cc_o[h].rearrange("(c j) e -> j c e", c=N_CORES)
            (dma_eng or nc.sync).dma_start(zh[:], src[ja - SEG[h][0] : jb - SEG[h][0]])
            sqs = s2t.tile([P, H], F32, tag=f"sqs{name}", name=f"sqs{name}")
            ssn = s2t.tile([P, 1], F32, tag=f"ssn{name}", name=f"ssn{name}")
            nc.vector.tensor_mul(sqs[:], zh[:], zh[:])
            nc.vector.reduce_sum(out=ssn[:], in_=sqs[:], axis=AX.X)
            # sqrt(TAU * ss): scales z by 1/sqrt(tau) so S = z'z'^T = logits
            nrm = s2t.tile([P, 1], F32, tag=f"nrm{name}", name=f"nrm{name}")
            nc.scalar.activation(nrm[:], ssn[:], AF.Sqrt, scale=TAU)
            rn = s2t.tile([P, 1], F32, tag=f"rn{name}", name=f"rn{name}")
            nc.vector.reciprocal(rn[:], nrm[:])
            nc.vector.tensor_scalar_mul(zh[:], zh[:], rn[:, 0:1])
            for k in range(6):
                pt = psT.tile([128, 128], F32, tag="pt")
                nc.tensor.transpose(
                    pt[:, 0:P], zh[:, k * 128 : (k + 1) * 128], idt[0:P, 0:P]
                )
                nc.vector.tensor_copy(
                    zT[:, k * B + col : k * B + col + P], pt[:, 0:P]
                )

        def logits_block(col, n):
            """S[0:64, col:col+n] += sum_k zT_k[:, 0:64].T @ zT_k[:, col:col+n]"""
            for k in range(6):
                nc.tensor.matmul(
                    pS[:, col : col + n],
                    lhsT=zT[:, k * B : k * B + N_PAIR],
                    rhs=zT[:, k * B + col : k * B + col + n],
                    start=(k == 0),
                    stop=(k == 5),
                )

        # ---- stage 1: per-batch sum over the sequence axis -------------------
        # partition p holds seq rows {c*128+p}: 4x 3KB DMA descriptors per
        # partition. (Measured FASTER than one 12KB descriptor per partition
        # - the smaller interleaved pattern spreads better across HBM.)
        x4 = x.rearrange("b (c p) e -> b p c e", p=128)  # [32, 128, 4, 768]
        for b in range(B_SH):
            if "s1" in stages:
                xt = xin.tile([128, 4 * H], F32R)
                nc.sync.dma_start(xt[:], x4[b])
                ps = ps1.tile([1, H], F32)
                for c in range(4):
                    nc.tensor.matmul(
                        ps[:, 0:512],
                        lhsT=ones_col[:, 0:1].bitcast(F32R),
                        rhs=xt[:, c * H : c * H + 512],
                        start=(c == 0),
                        stop=(c == 3),
                    )
                for c in range(4):
                    nc.tensor.matmul(
                        ps[:, 512:H],
                        lhsT=ones_col[:, 0:1].bitcast(F32R),
                        rhs=xt[:, c * H + 512 : (c + 1) * H],
                        start=(c == 0),
                        stop=(c == 3),
                    )
                nc.scalar.copy(pooled_sb[0:1, b * H : (b + 1) * H], ps[:])
            if "cc" in stages:
                # per-batch staging on the ACT HWDGE queue: never queued
                # behind the big SP-queue x stream
                nc.scalar.dma_start(
                    cc_in[b : b + 1, :], pooled_sb[0:1, b * H : (b + 1) * H]
                )
                for h, (j0, j1) in enumerate(SEG):
                    if b == j1 - 1:
                        send_seg(h)

        if "cc" not in stages or "s2" not in stages:
            return

        # ---- consume gathers 0,1 (hidden in the final gather's window) ------
        consume_block(0, 0, 16, "zb0")
        logits_block(0, 128)
        consume_block(1, 16, 24, "zb1")
        logits_block(128, 64)

        # ---- early finish: everything not needing columns [192,256) ---------
        # uv stacks [ld; rs] so one dot against cw = [cnt; -1] finishes it
        uv = s2.tile([128, 1], F32)
        dm = s2.tile([N_PAIR, 192], F32)
        nc.scalar.dma_start(dm[:], dmask[:, 0:192])
        sd0 = s2.tile([N_PAIR, 192], F32)
        nc.vector.tensor_add(sd0[:], pS[:, 0:192], dm[:])
        et0 = s2.tile([N_PAIR, 192], F32)
        se0 = s2.tile([N_PAIR, 1], F32)
        nc.scalar.activation(et0[:], sd0[:], AF.Exp, scale=1.0, accum_out=se0[:])
        tri_t = s2.tile([N_PAIR, N_PAIR], F32)
        nc.scalar.dma_start(tri_t[:], triu[:])
        mt2 = s2.tile([N_PAIR, N_PAIR], F32)
        nc.vector.tensor_mul(mt2[:], sd0[0:N_PAIR, 0:N_PAIR], tri_t[:])
        rs = s2.tile([N_PAIR, 1], F32)
        nc.vector.reduce_sum(out=rs[:], in_=mt2[:], axis=AX.X)
        # partition-shift rs into the bottom half of uv (SBUF->SBUF DMA)
        nc.gpsimd.dma_start(uv[N_PAIR : 2 * N_PAIR, 0:1], rs[:])
        cw_t = s2.tile([128, 1], F32)
        nc.scalar.dma_start(cw_t[:], cw[:])

        # ---- exposed tail: final gather block + short chain -----------------
        consume_block(2, 24, 32, "zb2", dma_eng=nc.gpsimd)
        logits_block(192, 64)
        # no diagonal in columns [192,256): exp straight from PSUM
        et1 = s2.tile([N_PAIR, 64], F32)
        se1 = s2.tile([N_PAIR, 1], F32)
        nc.scalar.activation(
            et1[:], pS[:, 192:256], AF.Exp, scale=1.0, accum_out=se1[:]
        )
        # logden = ln(se1 + se0): bias-AP fusion keeps the add off the tail
        nc.scalar.activation(uv[0:N_PAIR, :], se1[:], AF.Ln, bias=se0[:, 0:1])
        ptot = psS.tile([1, 1], F32, tag="ptot")
        nc.tensor.matmul(ptot[:], lhsT=uv[:], rhs=cw_t[:], start=True, stop=True)
        res = s2.tile([1, 1], F32)
        nc.vector.tensor_scalar_mul(res[:], ptot[:], -2.0 / N_PAIR * (N_PAIR - 1))
        nc.gpsimd.dma_start(out[0:1, 0:1], res[:])


def build_nc():
    nc = bacc.Bacc("TRN2", target_bir_lowering=False, debug=False, num_devices=N_CORES)
    x = nc.dram_tensor("x", [B_SH, S, H], X_DT, kind="ExternalInput")
    ident = nc.dram_tensor("ident", [128, 128], F32, kind="ExternalInput")
    dmask = nc.dram_tensor("dmask", [N_PAIR, B], F32, kind="ExternalInput")
    triu = nc.dram_tensor("triu", [N_PAIR, N_PAIR], F32, kind="ExternalInput")
    cw = nc.dram_tensor("cw", [128, 1], F32, kind="ExternalInput")
    out = nc.dram_tensor("loss", [1, 1], F32, kind="ExternalOutput")
    with tile.TileContext(nc) as tc:
        _body(
            tc,
            x.ap(),
            ident.ap(),
            dmask.ap(),
            triu.ap(),
            cw.ap(),
            out.ap(),
        )
    nc.compile()
    return nc


def const_inputs():
    ident = np.eye(128, dtype=np.float32)
    dmask = np.zeros((N_PAIR, B), dtype=np.float32)
    dmask[np.arange(N_PAIR), np.arange(N_PAIR)] = NEG
    triu = np.triu(np.ones((N_PAIR, N_PAIR), dtype=np.float32), k=1)
    cw = np.concatenate(
        [
            (N_PAIR - 1 - np.arange(N_PAIR, dtype=np.float32)),  # cnt_i
            -np.ones(N_PAIR, dtype=np.float32),
        ]
    ).reshape(128, 1)
    return {"ident": ident, "dmask": dmask, "triu": triu, "cw": cw}


def make_in_maps(last_hidden_states, input_mask):
    del input_mask  # cancels exactly in the L2 normalization (see send_seg)
    x = np.asarray(last_hidden_states, dtype=np.float32)
    consts = const_inputs()
    return [
        {"x": np.ascontiguousarray(x[c::N_CORES]), **consts}  # logical c+8j
        for c in range(N_CORES)
    ]


_CACHE = {}


def get_nc():
    if "nc" not in _CACHE:
        _CACHE["nc"] = build_nc()
    return _CACHE["nc"]


def kernel(last_hidden_states, input_mask):
    nc = get_nc()
    in_maps = make_in_maps(last_hidden_states, input_mask)
    res = bass_utils.run_bass_kernel_spmd(nc, in_maps, core_ids=list(range(N_CORES)))
    return np.asarray(res.results[0]["loss"], dtype=np.float32).reshape(())
